# revision 2
# baseline (speedup 1.0000x reference)
"""Trainium2 Bass kernel for nn_DrawImageLayer (draw Gaussian strokes, max over time).

Reference semantics:
  out[b,i,j,0] = min(1, max_t I[b,t] * exp(-g*(r_i - y[b,t])^2) * exp(-g*(r_j - x[b,t])^2))
  r_k = k/28 - 0.5, g = (28/2)^2 = 196, shapes B=1024, T=64, canvas 28x28.

Strategy: pure data parallel - 128 batch rows per NeuronCore (= SBUF
partitions) across 8 cores. Linear domain (no logs): I < 1 strictly, so the
min(.,1) clamp is a no-op and out = max_t I*py*px directly.

The DVE is the bottleneck engine, so everything big runs in fp16 with
t INNERMOST so every AP has a step-1 innermost dim -> DVE 2x_1P perf mode
(2 elem/cycle; broadcasts sit on outer dims where they don't block packing):
  V  d12[(h,k),t] = r_k - yx[t]      3584 elems, f32 in / fp16 out, 1x
  A  s12 = Square(sqrt_g * d12)      3584, fp16        (ACT is idle otherwise)
  A  e12 = Exp(-s12)                 3584, fp16        (py | px, k-major rows)
  A  i16 = Copy(I)                   64,  f32 -> fp16
  V  pyi[i,t] = e12y[i,t] * i16[t]   1792, fp16, 2x
  V  cube[(i,j),t] = pyi[i,t] * e12x[j,t]   50176, fp16, 2x  (outer-dim bcasts)
  V  img16[(i,j)] = reduce_max_t cube       50176 in, fp16, 2x
  A  img32 = Copy(img16)             784, fp16 -> f32
DVE steady state ~= (3584 + 1792/2 + 50176/2 + 50176/2 + overheads) cycles
~= 55k cycles ~= 57us @0.96GHz; ACT ~8us and DMA hide underneath.

Cross-rep software pipeline: V issues d12(k+1) right after pyi(k) so ACT
computes Square/Exp(k+1) during cube/reduce(k). Ping-pong buffers: xs, i16,
e12, img16, img32. Sync uses cumulative semaphore thresholds (dsx/dso for
DMA, vda/vra V->A, av/ac A->V/DMA); WAR hazards are discharged through
implication chains on the in-order engine queues (see comments).
"""

from contextlib import ExitStack

import numpy as np

import concourse.bass as bass
import concourse.mybir as mybir
from concourse.bass_utils import run_bass_kernel_spmd

SIZE = 28
T = 64
B = 1024
BC = 128  # batch rows per core
NCORES = 8
P2 = SIZE * SIZE  # 784
KT = SIZE * T  # 1792, one half of d12/e12 (k-major rows of T)
CUBE = P2 * T  # 50176
G = (SIZE / 2.0) ** 2
SQRT_G = float(np.sqrt(G))
F32 = mybir.dt.float32
F16 = mybir.dt.float16
AO = mybir.AluOpType
AF = mybir.ActivationFunctionType
RSOFF = T * 3  # grid columns appended after the (t,c) block
XCOLS = RSOFF + SIZE  # 220

_GRID = (np.arange(SIZE, dtype=np.float32) / SIZE - 0.5).astype(np.float32)


def _ap(t, offset, dims):
    """AP over an sbuf tensor: partition dim [row_pitch, 128] + free dims."""
    return bass.AP(t, offset, [[t.shape[1], BC]] + [list(d) for d in dims])


def build(rep: int = 1, drains: bool = False) -> bass.Bass:
    """One-core program, SPMD across 8 cores. rep>1 replicates the body
    (cumulative semaphore thresholds) for wall-clock delta timing."""
    nc = bass.Bass(detect_race_conditions=drains)
    xin = nc.declare_dram_parameter("xin", [BC, XCOLS], F32, isOutput=False)
    out = nc.declare_dram_parameter("out", [BC, P2], F32, isOutput=True)

    with ExitStack() as ctx:
        xs = ctx.enter_context(nc.sbuf_tensor([BC, 2 * XCOLS], F32))
        i16 = ctx.enter_context(nc.sbuf_tensor([BC, 2 * T], F16))
        d12 = ctx.enter_context(nc.sbuf_tensor([BC, 2 * KT], F16))
        s12 = ctx.enter_context(nc.sbuf_tensor([BC, 2 * KT], F16))
        e12 = ctx.enter_context(nc.sbuf_tensor([BC, 2 * 2 * KT], F16))
        pyi = ctx.enter_context(nc.sbuf_tensor([BC, KT], F16))
        cube = ctx.enter_context(nc.sbuf_tensor([BC, CUBE], F16))
        img16 = ctx.enter_context(nc.sbuf_tensor([BC, 2 * P2], F16))
        img32 = ctx.enter_context(nc.sbuf_tensor([BC, 2 * P2], F32))
        dsx = ctx.enter_context(nc.semaphore("dsx"))  # xs in-dma (+16 each)
        dso = ctx.enter_context(nc.semaphore("dso"))  # out-dma (+16 each)
        vda = ctx.enter_context(nc.semaphore("vda"))  # V d12(k) done -> k+1
        vra = ctx.enter_context(nc.semaphore("vra"))  # V reduce(k) done -> k+1
        av = ctx.enter_context(nc.semaphore("av"))  # A Exp(k) done -> k+1
        ac = ctx.enter_context(nc.semaphore("ac"))  # A Copy(k) done -> k+1
        block = ctx.enter_context(nc.Block())

        def d12_op(k):
            """d12[(h,k),t] = grid[k] - [y|x][t]; h=0 -> y (ch 1), h=1 -> x (ch 0)
            via the offset-1 step-(-1) channel trick. f32 in, fp16 out."""
            p = (k % 2) * XCOLS
            return nc.vector.tensor_tensor(
                _ap(d12, 0, [[KT, 2], [T, SIZE], [1, T]]),
                _ap(xs, p + RSOFF, [[0, 2], [1, SIZE], [0, T]]),
                _ap(xs, p + 1, [[-1, 2], [0, SIZE], [3, T]]),
                AO.subtract,
            )

        @block.sync
        def _(sync):
            for k in range(min(2, rep)):
                sync.dma_start(
                    out=_ap(xs, (k % 2) * XCOLS, [[1, XCOLS]]), in_=xin[:, :]
                ).then_inc(dsx, 16)
            for k in range(rep):
                if k + 2 < rep:
                    # xs[k%2] free once Exp(k) done (implies d12(k)+i16(k) read)
                    sync.dma_start(
                        out=_ap(xs, (k % 2) * XCOLS, [[1, XCOLS]]), in_=xin[:, :]
                    )._wait_ge(av, k + 1).then_inc(dsx, 16)
                sync.dma_start(
                    out=out[:, :], in_=_ap(img32, (k % 2) * P2, [[1, P2]])
                )._wait_ge(ac, k + 1).then_inc(dso, 16)
            sync.wait_ge(dsx, 16 * min(rep, 2) + 16 * max(0, rep - 2))
            sync.wait_ge(dso, rep * 16)

        @block.vector
        def _(vector):
            d12_op(0)._wait_ge(dsx, 16).then_inc(vda, 1)
            for k in range(rep):
                par = (k % 2) * 2 * KT
                # pyi[i,t] = py[i,t] * I[t]  (fp16 2x; waits Exp(k))
                nc.vector.tensor_tensor(
                    _ap(pyi, 0, [[T, SIZE], [1, T]]),
                    _ap(e12, par, [[T, SIZE], [1, T]]),
                    _ap(i16, (k % 2) * T, [[0, SIZE], [1, T]]),
                    AO.mult,
                )._wait_ge(av, k + 1)
                if k + 1 < rep:
                    # issue d12(k+1) now so ACT overlaps with cube/reduce(k).
                    # WAR on d12 vs Sq(k) is implied: pyi(k) waited Exp(k).
                    d12_op(k + 1)._wait_ge(dsx, 16 * (k + 2)).then_inc(vda, 1)
                # cube[(i,j),t] = pyi[i,t] * px[j,t]  (fp16 2x, bcasts on outer)
                nc.vector.tensor_tensor(
                    _ap(cube, 0, [[SIZE * T, SIZE], [T, SIZE], [1, T]]),
                    _ap(pyi, 0, [[T, SIZE], [0, SIZE], [1, T]]),
                    _ap(e12, par + KT, [[0, SIZE], [T, SIZE], [1, T]]),
                    AO.mult,
                )
                # img16 = max over t (innermost, fp16 2x)
                nc.vector.tensor_reduce(
                    _ap(img16, (k % 2) * P2, [[1, P2]]),
                    _ap(cube, 0, [[T, P2], [1, T]]),
                    mybir.AxisListType.X,
                    AO.max,
                ).then_inc(vra, 1)

        @block.scalar
        def _(scalar):
            for k in range(rep):
                # i16[t] = I[t] as fp16. WAR vs pyi(k-2) implied by Copy(k-2)'s
                # vra wait (reduce(k-2) follows pyi(k-2) on V's in-order queue).
                nc.scalar.activation(
                    _ap(i16, (k % 2) * T, [[1, T]]),
                    _ap(xs, (k % 2) * XCOLS + 2, [[3, T]]),
                    AF.Copy,
                )._wait_ge(dsx, 16 * (k + 1))
                nc.scalar.activation(
                    _ap(s12, 0, [[1, 2 * KT]]),
                    _ap(d12, 0, [[1, 2 * KT]]),
                    AF.Square,
                    scale=SQRT_G,
                )._wait_ge(vda, k + 1)
                nc.scalar.activation(
                    _ap(e12, (k % 2) * 2 * KT, [[1, 2 * KT]]),
                    _ap(s12, 0, [[1, 2 * KT]]),
                    AF.Exp,
                    scale=-1.0,
                ).then_inc(av, 1)
                # out-convert of the PREVIOUS rep, after Exp(k) so ACT never
                # blocks the V-critical Exp behind a reduce wait. img32 WAR vs
                # out-dma(k-3) is discharged by the 2-rep gap (and the final
                # rep's dma is strictly ordered via ac/dso).
                if k > 0:
                    nc.scalar.activation(
                        _ap(img32, ((k - 1) % 2) * P2, [[1, P2]]),
                        _ap(img16, ((k - 1) % 2) * P2, [[1, P2]]),
                        AF.Copy,
                    )._wait_ge(vra, k).then_inc(ac, 1)
            nc.scalar.activation(
                _ap(img32, ((rep - 1) % 2) * P2, [[1, P2]]),
                _ap(img16, ((rep - 1) % 2) * P2, [[1, P2]]),
                AF.Copy,
            )._wait_ge(vra, rep).then_inc(ac, 1)

    return nc


def make_in_maps(x: np.ndarray) -> list:
    """Shard x (1024, 64, 3) -> per-core maps; grid constant appended."""
    maps = []
    for c in range(NCORES):
        xc = x[c * BC : (c + 1) * BC].reshape(BC, T * 3).astype(np.float32)
        xc = np.concatenate([xc, np.broadcast_to(_GRID, (BC, SIZE))], axis=1)
        maps.append({"xin": np.ascontiguousarray(xc)})
    return maps


def kernel(x: np.ndarray) -> np.ndarray:
    """Full inputs in, full output out: (1024, 64, 3) f32 -> (1024, 28, 28, 1) f32."""
    x = np.asarray(x, dtype=np.float32)
    assert x.shape == (B, T, 3), x.shape
    nc = build(rep=1)
    res = run_bass_kernel_spmd(nc, make_in_maps(x), list(range(NCORES)))
    outs = [res.results[c]["out"].reshape(BC, SIZE, SIZE, 1) for c in range(NCORES)]
    return np.concatenate(outs, axis=0)
